# revision 34
# baseline (speedup 1.0000x reference)
"""Trainium2 Bass kernel for nn_AttLayer (sliding-block attention encoder layer).

Sharding: 8 cores = 4 batches x 2 sequence halves (4096 frames each).
Each core gets its x1 slice with a 256-frame halo on both sides (zero-padded at
sequence edges), computes q/k/v projections, 8 blocks of windowed attention
(block 512, window 1024), relu + output projection locally. No collectives.

Numerics: fp16 matmul inputs everywhere (x, W, q, k, v, p), fp32 PSUM
accumulation, fp32 biases/normalization.  Simulated end-to-end rel err ~8e-4
(vs 2e-2 budget).

Perf notes vs the fp32r baseline (200us -> this):
  - fp16 halves all DMA traffic and SBUF footprints; matmul rate is identical
    (1 cycle/row for fp32r/bf16/fp16 at FD>=256 on TRN2).
  - DMAs are PACKED: one DMA per x chunk ([128,4,512] 3D AP), all weights in
    one [128,4096] tensor, all biases + the softmax log-mask bias in one
    [128,74] f32 tensor.  The Sync engine issues one DMA per ~700ns, so the
    baseline's 26-DMA head serialized the first 18us of the kernel.
  - projection chunks are interleaved with attention blocks (block b only
    needs chunks <= b+1), so the PE never waits on a phase boundary.
  - softmax row-sum: fp16 pairwise adds on DVE (2x mode) + one ones-matmul,
    instead of 7 fp32 adds (39us -> 20us of DVE time).
  - q/k PSUM evacuation on the Scalar engine (Identity+bias), v/out evac on
    DVE: balances ACT ~8us, DVE ~8us per 12us PE step.
"""

import numpy as np

# problem constants (self-contained; must match the harness reference)
B, CIN, L = 4, 512, 8192
C, VD = 256, 512
BL, HALF = 512, 256
NCORES = 8
LCH = L // 2            # 4096 frames per core
LEXT = LCH + 2 * HALF   # 4608 with halo
NBLK = LCH // BL        # 8 local blocks
WS = BL + 2 * HALF      # 1024 window
NKT = WS // 128         # 8 k-tiles per window
NCH = LEXT // BL        # 9 x chunks
NVT = LEXT // 128       # 36 v^T partition tiles
NWARM = 17              # PE warmup matmuls (HAM clock-gate ramp)
NTAIL = 10              # PE filler matmuls bridging the last block's
                        # softmax-normalization latency (keeps HAM at 8/8)

_NC_CACHE = {}


def _build_nc():
    import concourse.bacc as bacc
    import concourse.mybir as mybir
    import concourse.tile as tile
    from contextlib import ExitStack

    f32 = mybir.dt.float32
    f16 = mybir.dt.float16
    AF = mybir.ActivationFunctionType

    nc = bacc.Bacc("TRN2", target_bir_lowering=False, debug=False,
                   num_devices=NCORES)

    # x is packed CHUNK-MAJOR on the host: row (c*128+p) holds the 4 cin
    # r-groups of chunk c, partition p, concatenated -> 4KB contiguous DMA
    # lines (1KB lines measured ~60 GB/s, 4KB ~300 GB/s).
    x_d = nc.dram_tensor("x", [NCH * 128, 4 * BL], f16,
                         kind="ExternalInput").ap()
    wpk_d = nc.dram_tensor("wpk", [128, 4096], f16, kind="ExternalInput").ap()
    fpk_d = nc.dram_tensor("fpk", [128, 74], f32, kind="ExternalInput").ap()
    out_d = nc.dram_tensor("out", [VD, LCH], f16, kind="ExternalOutput").ap()

    x_r = x_d.rearrange("(c p) w -> p c w", p=128)      # [128, 9, 2048]
    out_r = out_d.rearrange("(v p) l -> p v l", p=128)  # [128, 4, 4096]

    with tile.TileContext(nc) as tc:
        with ExitStack() as ctx:
            ctx.enter_context(nc.allow_low_precision(
                reason="fp16 matmul pipeline; fp32 PSUM accumulation"))
            sbc = ctx.enter_context(tc.tile_pool(name="sbc", bufs=1))  # consts
            sbp = ctx.enter_context(tc.tile_pool(name="sbp", bufs=1))  # persist
            sbs = ctx.enter_context(tc.tile_pool(name="sbs", bufs=1))  # stream
            ps = ctx.enter_context(tc.tile_pool(name="ps", bufs=1, space="PSUM"))

            dma = nc.sync.dma_start

            # Tile-framework dependencies are TILE-granular: a consumer waits
            # for ALL outstanding writes to a tile.  So the weight pack is
            # split into per-consumer tiles (wq / wk / wv|wo) and chunk 0 of
            # x into two halves -- the first q matmul then gates only on
            # wq + x0[r0,r1] (~0.6 MB) instead of the whole 1.5 MB head.
            wqs = sbc.tile([128, 1024], f16, tag="wqs", name="wqs")
            wks = sbc.tile([128, 1024], f16, tag="wks", name="wks")
            wvo = sbc.tile([128, 2048], f16, tag="wvo", name="wvo")
            fpk = sbc.tile([128, 74], f32, tag="fpk", name="fpk")
            xta = sbs.tile([128, 2 * BL], f16, tag="xta", name="xta")
            xtb = sbs.tile([128, 2 * BL], f16, tag="xtb", name="xtb")
            # SP DMA queue starts ~8.7us, ACT queue ~10.5us; each ~260 GB/s
            # with a slow ramp.  SP carries the first-needed pieces.
            dma(out=wqs[:], in_=wpk_d[0:128, 0:1024])
            dma(out=xta[:], in_=x_r[:, 0, 0:2 * BL])
            dma(out=xtb[:], in_=x_r[:, 0, 2 * BL:4 * BL])
            dma(out=wks[:], in_=wpk_d[0:128, 1024:2048])
            nc.scalar.dma_start(out=wvo[:], in_=wpk_d[0:128, 2048:4096])
            nc.scalar.dma_start(out=fpk[:], in_=fpk_d)

            # packed-weight / bias access helpers
            def wq_ap(r, o):
                return wqs[:, r * 256 + o * 128:r * 256 + (o + 1) * 128]

            def wk_ap(r, o):
                return wks[:, r * 256 + o * 128:r * 256 + (o + 1) * 128]

            def wv_ap(r):
                return wvo[:, r * 256:(r + 1) * 256]

            def wo_ap(m, v):
                return wvo[:, 1024 + m * 512 + v * 128:
                           1024 + m * 512 + (v + 1) * 128]

            def bq_ap(o):
                return fpk[:, o:o + 1]

            def bk_ap(o):
                return fpk[:, 2 + o:3 + o]

            def bv_ap(m):
                return fpk[:, 4 + m:5 + m]

            def bo_ap(v):
                return fpk[:, 6 + v:7 + v]

            def ab_ap(b, kt):
                return fpk[:, 10 + b * NKT + kt:11 + b * NKT + kt]

            # PE warmup: dependency-free matmuls during the DMA+preamble head
            # so the HAM clock-gate ramps before real work arrives.
            wrm = sbc.tile([128, BL], f16, tag="wrm", name="wrm")
            nc.gpsimd.memset(wrm[:], 0.0)
            # dummy activation: the first ACTIVATE triggers a 1.3us
            # ACT_TABLE_LOAD on the Scalar engine; take it in the head.
            wrs = sbc.tile([128, 1], f16, tag="wrs", name="wrs")
            nc.scalar.activation(wrs[:], wrm[:, 0:1], AF.Relu, bias=0.0,
                                 scale=1.0)
            wps = ps.tile([128, BL], f32, tag="pp", bufs=2, name="wps")
            for _ in range(NWARM):
                nc.tensor.matmul(wps[:], wrm[:, 0:128], wrm[:], start=True,
                                 stop=True)

            ones_k = sbc.tile([128, 1], f16, tag="ones_k", name="ones_k")
            nc.vector.memset(ones_k[:], 1.0)

            q2 = sbp.tile([128, 2, LEXT], f16, tag="q2", name="q2")
            k2 = sbp.tile([128, 2, LEXT], f16, tag="k2", name="k2")
            vts = [sbp.tile([128, C], f16, tag=f"vt{i}", name=f"vt{i}")
                   for i in range(NVT)]

            def mm(out_ap, lhsT, rhs, start, stop):
                nc.tensor.matmul(out_ap, lhsT, rhs, start=start, stop=stop)

            # ---------------- projections (one chunk = 512 frames) ----------
            XT = {}

            def emit_chunk(c):
                if c == 0:
                    def xs(r, lo, hi):
                        t = xta if r < 2 else xtb
                        return t[:, (r % 2) * BL + lo:(r % 2) * BL + hi]
                else:
                    if c in XT:
                        xt = XT[c]
                    else:
                        xt = sbs.tile([128, 4 * BL], f16, tag="x", bufs=3,
                                      name=f"xt{c}")
                        dma(out=xt[:], in_=x_r[:, c, :])
                        XT[c] = xt

                    def xs(r, lo, hi):
                        return xt[:, r * BL + lo:r * BL + hi]
                # q is only needed on extended cols [HALF, LEXT-HALF)
                qlo = max(c * BL, HALF) - c * BL
                qhi = min((c + 1) * BL, LEXT - HALF) - c * BL
                for o in range(2):
                    pq = ps.tile([128, BL], f32, tag="pp", bufs=2,
                                 name=f"pq{c}_{o}")
                    for r in range(4):
                        mm(pq[:, 0:qhi - qlo], wq_ap(r, o),
                           xs(r, qlo, qhi), r == 0, r == 3)
                    nc.scalar.activation(
                        q2[:, o, c * BL + qlo:c * BL + qhi],
                        pq[:, 0:qhi - qlo], AF.Identity, bias=bq_ap(o),
                        scale=1.0)
                if c == 0:
                    # bridge the wk DMA arrival so the HAM stays open
                    for _ in range(3):
                        nc.tensor.matmul(wps[:], wrm[:, 0:128], wrm[:],
                                         start=True, stop=True)
                for o in range(2):
                    pk = ps.tile([128, BL], f32, tag="pp", bufs=2,
                                 name=f"pk{c}_{o}")
                    for r in range(4):
                        mm(pk[:], wk_ap(r, o), xs(r, 0, BL), r == 0, r == 3)
                    nc.scalar.activation(
                        k2[:, o, c * BL:(c + 1) * BL], pk[:], AF.Identity,
                        bias=bk_ap(o), scale=1.0)
                for lt in range(4):
                    pv = ps.tile([128, C], f32, tag="pp", bufs=2,
                                 name=f"pv{c}_{lt}")
                    for r in range(4):
                        mm(pv[:], xs(r, lt * 128, (lt + 1) * 128), wv_ap(r),
                           r == 0, r == 3)
                    nc.vector.tensor_copy(vts[c * 4 + lt][:], pv[:])

            # ---------------- attention (software-pipelined blocks) ---------
            PTS, OPS, SPS, RBS, ORL = {}, {}, {}, {}, {}

            SARS = {}

            def emit_qk(b):
                pt = sbs.tile([128, NKT, BL], f16, tag="pt", bufs=2,
                              name=f"pt{b}")
                # softmax row-sum tree runs EAGERLY on DVE as the exp tiles
                # land, so the reciprocal is ready before AV finishes.
                ua = sbs.tile([128, 2, BL], f16, tag="ua", bufs=2,
                              name=f"ua{b}")
                ub = sbs.tile([128, 2, BL], f16, tag="ub", bufs=2,
                              name=f"ub{b}")
                uc = sbs.tile([128, 2, BL], f16, tag="uc", bufs=2,
                              name=f"uc{b}")
                sar = sbs.tile([128, BL], f16, tag="sar", bufs=2,
                               name=f"sar{b}")
                for kt in range(NKT):
                    pe = ps.tile([128, BL], f32, tag="e", bufs=3,
                                 name=f"e{b}_{kt}")
                    for ct in range(2):
                        mm(pe[:],
                           k2[:, ct, b * BL + kt * 128:b * BL + (kt + 1) * 128],
                           q2[:, ct, HALF + b * BL:HALF + (b + 1) * BL],
                           ct == 0, ct == 1)
                    nc.scalar.activation(pt[:, kt, :], pe[:], AF.Exp,
                                         bias=ab_ap(b, kt), scale=1.0 / 16.0)
                    if kt == 3:
                        nc.vector.tensor_add(ua[:], pt[:, 0:2, :],
                                             pt[:, 2:4, :])
                    elif kt == NKT - 1:
                        nc.vector.tensor_add(ub[:], pt[:, 4:6, :],
                                             pt[:, 6:8, :])
                        nc.vector.tensor_add(uc[:], ua[:], ub[:])
                        nc.vector.tensor_add(sar[:], uc[:, 0, :],
                                             uc[:, 1, :])
                PTS[b] = pt
                SARS[b] = sar

            def emit_av(b):
                pt = PTS[b]
                o0 = ps.tile([128, BL], f32, tag="o0", bufs=1, name=f"o0_{b}")
                o1 = ps.tile([128, BL], f32, tag="o1", bufs=1, name=f"o1_{b}")
                for kt in range(NKT):
                    vt = vts[b * 4 + kt]
                    mm(o0[:], vt[:, 0:128], pt[:, kt, :], kt == 0,
                       kt == NKT - 1)
                    mm(o1[:], vt[:, 128:256], pt[:, kt, :], kt == 0,
                       kt == NKT - 1)
                    if kt == 3:
                        # partition-reduce the row sums mid-AV so the
                        # recip -> broadcast chain overlaps the AV tail
                        sp = ps.tile([1, BL], f32, tag="s", bufs=1,
                                     name=f"s{b}")
                        mm(sp[:], ones_k[:], SARS[b][:], True, True)
                        SPS[b] = sp
                OPS[b] = (o0, o1)

            def emit_finA(b):
                rc = sbs.tile([1, BL], f32, tag="rc", bufs=2, name=f"rc{b}")
                nc.vector.reciprocal_approx_fast(rc[:], SPS[b][:])
                rb = sbs.tile([128, BL], f32, tag="rbs", bufs=2, name=f"rb{b}")
                nc.gpsimd.partition_broadcast(rb[:], rc[:])
                RBS[b] = rb

            def emit_normrelu(b):
                orl = []
                for m in range(2):
                    on = sbs.tile([128, BL], f32, tag=f"on{m}", bufs=2,
                                  name=f"on{b}_{m}")
                    nc.vector.tensor_mul(on[:], OPS[b][m][:], RBS[b][:])
                    rl = sbs.tile([128, BL], f16, tag=f"rl{m}", bufs=2,
                                  name=f"rl{b}_{m}")
                    nc.scalar.activation(rl[:], on[:], AF.Relu,
                                         bias=bv_ap(m), scale=1.0)
                    orl.append(rl)
                ORL[b] = orl

            def emit_outproj(b):
                ob = sbs.tile([128, 4, BL], f16, tag="ob", bufs=2,
                              name=f"ob{b}")
                for v in range(4):
                    po = ps.tile([128, BL], f32, tag="pp", bufs=2,
                                 name=f"po{b}_{v}")
                    for m in range(2):
                        mm(po[:], wo_ap(m, v), ORL[b][m][:], m == 0, m == 1)
                    # alternate evac engines so the four quarters drain in
                    # ~2 slots instead of 4 serial DVE ops
                    if v % 2 == 0:
                        nc.scalar.activation(ob[:, v, :], po[:], AF.Identity,
                                             bias=bo_ap(v), scale=1.0)
                    else:
                        nc.vector.tensor_scalar_add(ob[:, v, :], po[:],
                                                    bo_ap(v))
                    # per-quarter DMA: overlaps the remaining evacs
                    dma(out=out_r[:, v, b * BL:(b + 1) * BL],
                        in_=ob[:, v, :])

            # main software pipeline: chunks interleave with blocks
            # (block b needs chunks <= b+1 only).  outproj(b-1) is emitted
            # BEFORE av(b) so the DVE order is mul, ob-evacs, rowsum-tree,
            # recip -- keeping the reciprocal off the per-block critical path.
            emit_chunk(0)
            emit_chunk(1)
            for b in range(NBLK + 1):
                if b == NBLK:
                    # dependency-free filler matmuls: the PE would otherwise
                    # idle through the last block's mul->relu latency and the
                    # HAM clock-gate would drop to 4/8 right before the final
                    # output projection.
                    wpt = ps.tile([128, BL], f32, tag="e", bufs=3,
                                  name="wpst")
                    for _ in range(NTAIL):
                        nc.tensor.matmul(wpt[:], wrm[:, 0:128], wrm[:],
                                         start=True, stop=True)
                if b >= 1:
                    emit_normrelu(b - 1)
                if b < NBLK:
                    emit_qk(b)
                if b >= 1:
                    emit_outproj(b - 1)
                if b < NBLK:
                    emit_av(b)
                    emit_finA(b)
                if b + 2 < NCH:
                    emit_chunk(b + 2)

    nc.compile()
    return nc


def get_nc():
    key = "fp16"
    if key not in _NC_CACHE:
        _NC_CACHE[key] = _build_nc()
    return _NC_CACHE[key]


def make_core_inputs(inputs):
    """Split full inputs into 8 per-core input maps."""
    x1 = np.asarray(inputs["x1"], dtype=np.float32)
    mask = np.asarray(inputs["mask"], dtype=np.float32)
    wq_t = np.asarray(inputs["Wq"], np.float32).T.astype(np.float16)
    wk_t = np.asarray(inputs["Wk"], np.float32).T.astype(np.float16)
    wv_t = np.asarray(inputs["Wv"], np.float32).T.astype(np.float16)
    wo_t = np.asarray(inputs["Wo"], np.float32).T.astype(np.float16)
    bq = np.asarray(inputs["bq"], np.float32).reshape(C)
    bk = np.asarray(inputs["bk"], np.float32).reshape(C)
    bv = np.asarray(inputs["bv"], np.float32).reshape(C)
    bo = np.asarray(inputs["bo"], np.float32).reshape(VD)

    # packed weights [128, 4096] fp16: wq | wk | wv (4 r-tiles x 256 each),
    # wo (2 m-tiles x 512)
    wpk = np.empty((128, 4096), np.float16)
    for r in range(4):
        wpk[:, r * 256:(r + 1) * 256] = wq_t[r * 128:(r + 1) * 128, :]
        wpk[:, 1024 + r * 256:1024 + (r + 1) * 256] = \
            wk_t[r * 128:(r + 1) * 128, :]
        wpk[:, 2048 + r * 256:2048 + (r + 1) * 256] = \
            wv_t[r * 128:(r + 1) * 128, :]
    for m in range(2):
        wpk[:, 3072 + m * 512:3072 + (m + 1) * 512] = \
            wo_t[m * 128:(m + 1) * 128, :]
    wpk = np.ascontiguousarray(wpk)

    # padded log-mask (the reference pads mask with zeros, then adds
    # log(mask + 1e-6) to the energies)
    mp = np.pad(mask[:, 0, :], ((0, 0), (HALF, HALF)))
    lb = np.log(mp + np.float32(1e-6)).astype(np.float32)  # [B, L + 2*HALF]

    x16 = x1.astype(np.float16)

    in_maps = []
    for core in range(NCORES):
        b, h = divmod(core, 2)
        s = h * LCH
        xe = np.zeros((CIN, LEXT), np.float16)
        lo, hi = s - HALF, s + LCH + HALF
        slo, shi = max(lo, 0), min(hi, L)
        xe[:, slo - lo:slo - lo + (shi - slo)] = x16[b, :, slo:shi]
        # chunk-major pack: row (c*128+p) = [r0|r1|r2|r3] of chunk c,
        # partition p (4KB contiguous DMA lines)
        xcm = np.ascontiguousarray(
            xe.reshape(4, 128, NCH, BL).transpose(2, 1, 0, 3)
            .reshape(NCH * 128, 4 * BL))
        # packed f32 tensor: bq(2) | bk(2) | bv(2) | bo(4) | abias(64)
        fpk = np.empty((128, 74), np.float32)
        fpk[:, 0] = bq[0:128]
        fpk[:, 1] = bq[128:256]
        fpk[:, 2] = bk[0:128]
        fpk[:, 3] = bk[128:256]
        fpk[:, 4] = bv[0:128]
        fpk[:, 5] = bv[128:256]
        for v in range(4):
            fpk[:, 6 + v] = bo[v * 128:(v + 1) * 128]
        for blk in range(NBLK):
            w = lb[b, s + blk * BL:s + blk * BL + WS]
            fpk[:, 10 + blk * NKT:10 + (blk + 1) * NKT] = \
                w.reshape(NKT, 128).T
        in_maps.append({"x": xcm, "wpk": wpk, "fpk": fpk})
    return in_maps


def assemble_output(results):
    out = np.empty((B, VD, L), np.float32)
    for core in range(NCORES):
        b, h = divmod(core, 2)
        out[b, :, h * LCH:(h + 1) * LCH] = \
            results[core]["out"].astype(np.float32)
    return out


LAST_RESULT = None


def kernel(**inputs):
    global LAST_RESULT
    from concourse.bass_utils import run_bass_kernel_spmd

    nc = get_nc()
    in_maps = make_core_inputs(inputs)
    res = run_bass_kernel_spmd(nc, in_maps, list(range(NCORES)))
    LAST_RESULT = res
    return assemble_output(res.results)
